# revision 20
# baseline (speedup 1.0000x reference)
"""SSC keypoint-NMS kernel for Trainium2 (8 NeuronCores, SPMD).

Algorithm: the reference scatters 10000 scored keypoints into a dense
2160x3840 map and binary-searches a max-pool window size k until the
count of local maxima lands in [461, 563].  A pixel survives window k
iff no strictly-stronger pixel lies within Chebyshev distance k//2, so
it suffices to compute, per point, the Chebyshev distance to the
nearest stronger point: d(i) = min_{j<i} max(|y_i-y_j|, |x_i-x_j|)
(points sorted by descending score).  The whole binary search then
collapses to threshold counts over d, evaluated on host.

Device does the O(N^2) part (~50M ordered pairs) sharded over 8 cores.
Per 128x1024 pair-block the PE computes difference matrices
s = u_i - u_j, t = v_i - v_j of rotated coords u = y+x, v = y-x
(2*cheby = |s|+|t|) via K=4 bf16 matmuls (hi/lo byte split keeps 13-bit
coordinates exact at full PE rate); ACT (abs), Pool (pair-add) and DVE
(fused abs-add from PSUM / fp16 adds / min-reduce) split the
elementwise work in a statically balanced mix.

The triangular i<j structure is SPMD-uniform: the 80 i-tiles are
bucketed into 10 slots (slot t = tiles g = 8t+c, which need exactly
t+1 1024-wide j-chunks), so every core runs an identical 55-item
program; per-core variation (tile coords, the final partially-masked
chunk, its mask) lives in host-prepared input data.  All compute
operands are [4, w] / [128, w] APs at partition base 0 (ISA restricts
operand partition base to {0, 32, 64}).  Intermediates are fp16, exact
below 2048 = more than the 2*481 range the binary search can probe.
"""
import os
import sys
import time

sys.path.insert(0, "/opt/trn_rl_repo")
import numpy as np

_H, _W, _N = 2160, 3840, 10000
_NPAD = 10240
_NCORES = 8
_NSLOT = 10
_NFULL = 45                              # sum(t) for t in 0..9, 1024-wide
_N_ITEMS = _NFULL + _NSLOT               # + one masked item per slot
_FULL_BASE = [t * (t - 1) // 2 for t in range(_NSLOT)]
_SENT = float(2 ** 19)   # padded-point coordinate; bf16-exact after hi/lo split
_MASKV = 32768.0         # added to j>=i / padding entries; fp16-exact
_TOL = 0.1

LAST_EXEC_NS = None
_PROG = None


def _path(half):
    """Static engine mix per 512-wide half-item (110 total), tuned
    against the instruction cost model so ACT/Pool/DVE all land at
    ~94us: B = pure-DVE (fused abs-add from PSUM), C = ACT abs + DVE
    fp16 add, A = ACT abs + Pool fp16 add.  Masked halves (>= 2*_NFULL)
    stay A/C (the mask add needs the abs tile)."""
    masked = half >= 2 * _NFULL
    if not masked and half % 5 == 0:
        return "B"
    if half % 13 == 3:
        return "C"
    return "A"


def _core_tiles(c):
    """slot t -> global i-tile g; tile g needs ceil((g+1)/8) = t+1 chunks."""
    return [8 * t + c for t in range(_NSLOT)]


def _items():
    """Program item order, shared by the device builder and the host
    column grouping: full chunks slot-major, masked items last (after
    their 2.6MB mask DMA has long completed; interleaving them earlier
    was measured slower on the cost-model timeline)."""
    full = [(t, r) for t in range(_NSLOT) for r in range(t)]
    masked = [(t, None) for t in range(_NSLOT)]
    return full + masked


_NC = None


def _build_program():
    global _NC
    if _NC is not None:
        return _NC
    import concourse.bacc as bacc
    import concourse.mybir as mybir
    import concourse.tile as tile

    f32 = mybir.dt.float32
    bf16 = mybir.dt.bfloat16
    fp16 = mybir.dt.float16
    Alu = mybir.AluOpType
    Act = mybir.ActivationFunctionType
    X = mybir.AxisListType.X

    nc = bacc.Bacc("TRN2", target_bir_lowering=False, debug=False, num_devices=_NCORES)
    lhs = nc.declare_dram_parameter("lhs", [4, 256 * _NSLOT], bf16, isOutput=False)
    rhs_u = nc.declare_dram_parameter("rhs_u", [4, _NPAD], bf16, isOutput=False)
    rhs_v = nc.declare_dram_parameter("rhs_v", [4, _NPAD], bf16, isOutput=False)
    rhs_m = nc.declare_dram_parameter("rhs_m", [4, 2048 * _NSLOT], bf16, isOutput=False)
    maskp = nc.declare_dram_parameter("mask", [128, 1024 * _NSLOT], fp16, isOutput=False)
    dcol = nc.declare_dram_parameter("dcol", [128, 128], f32, isOutput=True)

    with tile.TileContext(nc) as tc:
        with (
            tc.tile_pool(name="const", bufs=1) as cpool,
            tc.tile_pool(name="ps", bufs=4, space="PSUM") as pspool,
            tc.tile_pool(name="ab", bufs=4) as apool,
            tc.tile_pool(name="mc", bufs=8) as mpool,
        ):
            lhs_t = cpool.tile([4, 256 * _NSLOT], bf16, tag="lhs")
            ru_t = cpool.tile([4, _NPAD], bf16, tag="ru")
            rv_t = cpool.tile([4, _NPAD], bf16, tag="rv")
            rm_t = cpool.tile([4, 2048 * _NSLOT], bf16, tag="rm")
            mk_t = cpool.tile([128, 1024 * _NSLOT], fp16, tag="mk")
            col_t = cpool.tile([128, 128], f32, tag="col")
            nc.sync.dma_start(lhs_t[:], lhs[:])
            nc.sync.dma_start(ru_t[:], rhs_u[:])
            nc.sync.dma_start(rv_t[:], rhs_v[:])
            nc.sync.dma_start(rm_t[:], rhs_m[:])
            nc.sync.dma_start(mk_t[:], maskp[:])
            nc.gpsimd.memset(col_t[:], 0.0)

            # each item runs as two 512-wide halves through 2-bank PSUM
            # tiles so four drains can be in flight (col 2m+h per half,
            # host mins the pair)
            for m, (t, r) in enumerate(_items()):
                lu = lhs_t[0:4, 256 * t:256 * t + 128]
                lv = lhs_t[0:4, 256 * t + 128:256 * t + 256]
                for h in range(2):
                    path = _path(2 * m + h)
                    o = 512 * h
                    if r is not None:
                        ru = ru_t[0:4, 1024 * r + o:1024 * r + o + 512]
                        rv = rv_t[0:4, 1024 * r + o:1024 * r + o + 512]
                    else:
                        ru = rm_t[0:4, 2048 * t + o:2048 * t + o + 512]
                        rv = rm_t[0:4, 2048 * t + 1024 + o:2048 * t + 1024 + o + 512]

                    ps_t = pspool.tile([128, 1024], f32, tag="ps")
                    nc.tensor.matmul(ps_t[:, 0:512], lu, ru, start=True, stop=True)
                    nc.tensor.matmul(ps_t[:, 512:1024], lv, rv, start=True, stop=True)

                    mc_t = mpool.tile([128, 512], fp16, tag="mc")
                    if path == "B":
                        pv = ps_t[:].rearrange("q (two n) -> q n two", two=2)
                        with nc.allow_low_precision(reason="2-term |s|+|t|; exact below 2048 which covers the decision range"):
                            nc.vector.tensor_reduce(
                                out=mc_t[:], in_=pv, axis=X, op=Alu.add,
                                apply_absolute_value=True)
                    else:
                        ab_t = apool.tile([128, 1024], fp16, tag="ab")
                        nc.scalar.activation(ab_t[:], ps_t[:], Act.Abs)
                        if r is None:
                            nc.vector.tensor_add(
                                ab_t[:, 0:512], ab_t[:, 0:512],
                                mk_t[:, 1024 * t + o:1024 * t + o + 512])
                        if path == "A":
                            nc.gpsimd.tensor_add(
                                mc_t[:], ab_t[:, 0:512], ab_t[:, 512:1024])
                        else:
                            nc.vector.tensor_add(
                                mc_t[:], ab_t[:, 0:512], ab_t[:, 512:1024])
                    nc.vector.tensor_reduce(
                        out=col_t[:, 2 * m + h:2 * m + h + 1], in_=mc_t[:],
                        axis=X, op=Alu.min)

            nc.sync.dma_start(dcol[:], col_t[:])

    nc.compile()
    _NC = nc
    return nc


def _split_hi_lo(a):
    """a -> (hi, lo) with a = 256*hi + lo, both bf16-exact for |a| <= 2^19."""
    hi = np.floor(a / 256.0)
    lo = a - 256.0 * hi
    return hi, lo


def _core_masks():
    """Input-independent per-core fp16 masks for the 10 masked chunks.
    Masked chunk of slot t covers j in [1024t, 1024(t+1)); its i-tile is
    g = 8t+c (ib = 1024t + 128c), so mask[p, q] = MASKV iff q >= 128c+p."""
    q = np.arange(1024)[None, :]
    p = np.arange(128)[:, None]
    masks = []
    for c in range(_NCORES):
        m = np.where(q >= 128 * c + p, _MASKV, 0.0).astype(np.float16)
        masks.append(np.tile(m, (1, _NSLOT)))
    return masks


def _build_runner():
    """Compile the Bass program once and wrap it in a persistent jitted
    SPMD executor (mirrors bass2jax.run_bass_via_pjrt, but reusable so
    repeat calls skip retracing/compiling)."""
    import jax
    from jax.experimental.shard_map import shard_map
    from jax.sharding import Mesh, NamedSharding, PartitionSpec
    from concourse import bass2jax
    import concourse.mybir as mybir

    nc = _build_program()
    bass2jax.install_neuronx_cc_hook()
    partition_name = nc.partition_id_tensor.name if nc.partition_id_tensor else None
    in_names, out_names, out_avals, out_shapes = [], [], [], []
    for alloc in nc.m.functions[0].allocations:
        if not isinstance(alloc, mybir.MemoryLocationSet):
            continue
        name = alloc.memorylocations[0].name
        if alloc.kind == "ExternalInput":
            if name != partition_name:
                in_names.append(name)
        elif alloc.kind == "ExternalOutput":
            out_names.append(name)
            shape = tuple(alloc.tensor_shape)
            dtype = mybir.dt.np(alloc.dtype)
            out_avals.append(jax.core.ShapedArray(shape, dtype))
            out_shapes.append((shape, dtype))
    n_params = len(in_names)
    n_outs = len(out_avals)
    all_in_names = list(in_names) + list(out_names)
    if partition_name is not None:
        all_in_names.append(partition_name)
    donate = tuple(range(n_params, n_params + n_outs))

    def _body(*args):
        operands = list(args)
        if partition_name is not None:
            operands.append(bass2jax.partition_id_tensor())
        outs = bass2jax._bass_exec_p.bind(
            *operands,
            out_avals=tuple(out_avals),
            in_names=tuple(all_in_names),
            out_names=tuple(out_names),
            lowering_input_output_aliases=(),
            sim_require_finite=True,
            sim_require_nnan=True,
            nc=nc,
        )
        return tuple(outs)

    devices = jax.devices()[:_NCORES]
    mesh = Mesh(np.asarray(devices), ("core",))
    sharded = jax.jit(
        shard_map(
            _body, mesh=mesh,
            in_specs=(PartitionSpec("core"),) * (n_params + n_outs),
            out_specs=(PartitionSpec("core"),) * n_outs,
            check_rep=False,
        ),
        donate_argnums=donate,
        keep_unused=True,
    )
    sharding = NamedSharding(mesh, PartitionSpec("core"))
    # masks never change between calls: commit them to the mesh once
    mask_cat = jax.device_put(np.concatenate(_core_masks(), axis=0), sharding)

    def run(in_maps):
        concat = []
        for nm in in_names:
            if nm == "mask":
                concat.append(mask_cat)
            else:
                concat.append(
                    np.concatenate([np.asarray(m[nm]) for m in in_maps], axis=0))
        zeros = [np.zeros((_NCORES * s[0], *s[1:]), dt) for s, dt in out_shapes]
        outs = sharded(*concat, *zeros)
        return [
            {nm: np.asarray(outs[i]).reshape(_NCORES, *out_shapes[i][0])[c]
             for i, nm in enumerate(out_names)}
            for c in range(_NCORES)
        ]

    return run


def _get_runner():
    global _PROG
    if _PROG is None:
        _PROG = _build_runner()
    return _PROG


def _device_min_cheby(ys, xs):
    """d2[i] = min_{j<i} 2*cheby((ys,xs)[i], (ys,xs)[j]); huge if none."""
    import ml_dtypes

    global LAST_EXEC_NS
    run = _get_runner()
    bf = ml_dtypes.bfloat16
    n = len(ys)
    up = np.full(_NPAD, _SENT)
    up[:n] = (ys + xs).astype(np.float64)
    vp = np.full(_NPAD, _SENT)
    vp[:n] = (ys - xs).astype(np.float64)
    uh, ul = _split_hi_lo(up)
    vh, vl = _split_hi_lo(vp)

    ru = np.empty((4, _NPAD), np.float32)
    ru[0] = 256.0
    ru[1] = 1.0
    ru[2] = -256.0 * uh
    ru[3] = -ul
    rv = np.empty((4, _NPAD), np.float32)
    rv[0] = 256.0
    rv[1] = 1.0
    rv[2] = -256.0 * vh
    rv[3] = -vl
    ru = ru.astype(bf)
    rv = rv.astype(bf)

    in_maps = []
    metas = []
    for c in range(_NCORES):
        tiles = _core_tiles(c)
        lhs = np.ones((4, 256 * _NSLOT), np.float32)
        rm = np.empty((4, 2048 * _NSLOT), np.float32)
        rm[0] = 256.0
        rm[1] = 1.0
        for t, g in enumerate(tiles):
            ib = 128 * g
            sl = slice(256 * t, 256 * t + 128)
            lhs[0, sl] = uh[ib:ib + 128]
            lhs[1, sl] = ul[ib:ib + 128]
            sl = slice(256 * t + 128, 256 * t + 256)
            lhs[0, sl] = vh[ib:ib + 128]
            lhs[1, sl] = vl[ib:ib + 128]
            j0 = 1024 * t
            rm[2, 2048 * t:2048 * t + 1024] = -256.0 * uh[j0:j0 + 1024]
            rm[3, 2048 * t:2048 * t + 1024] = -ul[j0:j0 + 1024]
            rm[2, 2048 * t + 1024:2048 * (t + 1)] = -256.0 * vh[j0:j0 + 1024]
            rm[3, 2048 * t + 1024:2048 * (t + 1)] = -vl[j0:j0 + 1024]
        in_maps.append({"lhs": lhs.astype(bf), "rhs_u": ru, "rhs_v": rv,
                        "rhs_m": rm.astype(bf)})
        metas.append(tiles)

    t0 = time.perf_counter()
    last_err = None
    for attempt in range(3):
        try:
            results = run(in_maps)
            break
        except Exception as e:  # transient NRT/axon faults clear on retry
            last_err = e
            time.sleep(2.0)
    else:
        raise last_err
    wall_ns = (time.perf_counter() - t0) * 1e9
    LAST_EXEC_NS = wall_ns

    d2 = np.full(_NPAD, np.inf)
    for c in range(_NCORES):
        dcol_np = results[c]["dcol"].astype(np.float64)
        item_id = {it: m for m, it in enumerate(_items())}
        for t, g in enumerate(metas[c]):
            ib = 128 * g
            ids = [item_id[(t, r)] for r in range(t)] + [item_id[(t, None)]]
            cols = [2 * i + h for i in ids for h in range(2)]
            d2[ib:ib + 128] = dcol_np[:, cols].min(axis=1)
    return d2[:n]


def kernel(keypoints, scores, num_ret_points=512, rows=2160, cols=3840):
    kp = np.asarray(keypoints)
    sc = np.asarray(scores)
    n = kp.shape[0]
    assert n == _N, f"kernel hardcoded for N={_N}, got {n}"
    num_ret = int(num_ret_points)
    rows_i = int(rows)
    cols_i = int(cols)

    y = kp[:, 2].astype(np.int64)
    x = kp[:, 3].astype(np.int64)
    order = np.argsort(-sc, kind="stable")
    ys, xs, ss = y[order], x[order], sc[order]

    d = _device_min_cheby(ys, xs) * 0.5

    # strictly-greater semantics for tied scores (device treats earlier
    # sorted index as stronger; re-derive d for tie-group members)
    eq_prev = np.empty(n, bool)
    eq_prev[0] = False
    np.equal(ss[1:], ss[:-1], out=eq_prev[1:])
    if eq_prev.any():
        grp_start = np.arange(n)
        for i in range(1, n):
            if eq_prev[i]:
                grp_start[i] = grp_start[i - 1]
        for i in np.flatnonzero(eq_prev):
            g = grp_start[i]
            if g == 0:
                d[i] = np.inf
            else:
                d[i] = np.maximum(np.abs(ys[:g] - ys[i]),
                                  np.abs(xs[:g] - xs[i])).min()

    # pixel-level survival: winner = first (strongest) point at a pixel
    pix = ys * cols_i + xs
    _, first_idx, inv = np.unique(pix, return_index=True, return_inverse=True)
    d_pix = d[first_idx]       # per unique pixel
    d_eff = d_pix[inv]         # per sorted point

    # replicate the reference's host-synchronized binary search over k
    k_min = int(round(num_ret * (1 - _TOL)))
    k_max = int(round(num_ret * (1 + _TOL)))
    low = 1
    high = max(1, int(max(rows_i, cols_i) // max(1, int((n / max(1, num_ret)) ** 0.5))))
    prev_k = -1
    sel_h = None
    while True:
        k = (low + high) // 2
        if k == prev_k or low > high:
            break
        h = k // 2            # centered window keff (k made odd) has radius k//2
        cnt = int((d_pix > h).sum())
        if k_min <= cnt <= k_max:
            sel_h = h
            break
        elif cnt < k_min:
            high = k - 1
        else:
            low = k + 1
        prev_k = k
    if sel_h is None:
        kk = prev_k if prev_k > 0 else 1
        sel_h = kk // 2

    keep_sorted = d_eff > sel_h
    keep = np.empty(n, bool)
    keep[order] = keep_sorted
    sel = np.flatnonzero(keep)
    m_ = sel.size
    if m_ < num_ret:
        msk = np.zeros(n, bool)
        msk[sel] = True
        remaining = np.flatnonzero(~msk)
        sel = np.concatenate([sel, remaining[:num_ret - m_]])
    elif m_ > num_ret:
        sel = sel[:num_ret]
    return kp[sel], sc[sel]
